# revision 41
# baseline (speedup 1.0000x reference)
"""Multi-head attention (B=2, S=2048, D=768, H=12) on 8 TRN2 NeuronCores.

Sharding: data-parallel over batch x tensor-parallel over heads.
  core c -> batch c//4, heads 3*(c%4) .. 3*(c%4)+2
Each core computes its 3 heads end-to-end plus the partial output
projection (its 192 rows of w_proj). Host sums the 4 bf16 partials per
batch and adds b_proj + b_v @ w_proj (the V bias is folded out of the
device kernel entirely). No cross-core collectives needed.

Design (v6) — the kernel is an exp-throughput problem: softmax exp is
3*S*S = 12.6M elements on the one ACT engine (~107us), so everything
else is scheduled around keeping ACT saturated from ~25us onward:

  * All inputs host-cast to bf16 and DMA'd DIRECTLY into their SBUF
    tiles (no staging copies): each matmul carries one DMA-queue
    semaphore wait on its LDWEIGHTS (stationary) or MATMUL (moving)
    instruction. x goes via the ACT HWDGE ring, weights via the SP ring.
  * Attention units (qb, h) run pairwise-interleaved; the two streams'
    K=64 score matmuls are emitted adjacently and run concurrently in
    disjoint PE row-groups where their base partitions differ.
  * Everything that is not a score/AV matmul (K/Q/V projections, softmax
    denominator broadcast, output projection halves) is queued into a
    static one-use-per-group slot schedule through a shared 2-buffer
    PSUM pool, so the in-order PE queue never blocks on a slow
    dependency and HAM stays at full clock.
  * Normalize is split: a raw [65,512] PSUM->SBUF copy at pair end frees
    the AV accumulator bank immediately; the broadcast/reciprocal/scale
    runs one pair later from SBUF in its slot.
  * PE warmup matmuls + a dummy activation at t=0 pre-warm the HAM clock
    gate and pre-load the exp table set.

Device kernel layout notes (per core):
  xT   (768, 2048)  = x[b]^T           -> contraction dim on partitions
  wqk  (768, 384)   = [Wq_heads*SCALE | Wk_heads], cols q0 q1 q2 k0 k1 k2
  Q^T/K^T tiles: tq0=[q0;q1] (128p), tq1=[q2] (64p), tk0=[k0;k1], tk1=[k2]
    so each head's Q^T and K^T slices share a base partition (0 or 64),
    which the matmul tile_position inference requires.
  V1   per k-chunk [128, 195]: cols [v_h0 1 v_h1 1 v_h2 1] (ones col per head)
  Scores are computed transposed (k on partitions): S^T[k, qb] so that
  exp(S^T) feeds the AV matmul directly (no transpose of the softmax).
  AV: O'[65, qb] = [V_h|1]^T @ exp(S^T chunk); row 64 = softmax denominator.
  Normalize: recip(denom) broadcast over partitions via a K=1 ones-matmul.
"""

from contextlib import ExitStack

import ml_dtypes
import numpy as np

import concourse.bass as bass
import concourse.mybir as mybir
import concourse.tile as tile
from concourse import bacc
from concourse.bass_utils import run_bass_kernel_spmd

B, S, D = 2, 2048, 768
H, HD = 12, 64
SCALE = HD**-0.5
NCORES = 8
HPC = 3  # heads per core
P = 128
KD = D // P  # 6 chunks of contraction dim for projections
QB = 512  # query block (free dim of score matmuls)
NQB = S // QB  # 4
NKC = S // P  # 16 key chunks
GS = 2  # key chunks per exp group ([128, GS*QB] activation)
F32 = mybir.dt.float32
BF16 = mybir.dt.bfloat16
EXP = mybir.ActivationFunctionType.Exp
BF16NP = ml_dtypes.bfloat16

_CACHE = {}

# pairwise unit schedule: (qb, h) streams A/B per pair. The first unit of
# each pair reads Q^T/K^T at base partition 0, the second at base 64, so
# EVERY pair's K=64 score matmuls run concurrently in disjoint PE row
# groups (2x score throughput). This needs q1/k1 present at both bases:
# T1=[q0;q1], T2=[q1;q2], K1=[k0;k1], K2=[k1;k2], where the q1/k1 rows of
# T2/K2 are partition-shifted copies made by SBUF->SBUF DMA (no PE cost).
# Query blocks complete in order qb0..qb3 so output projections spread
# across the whole kernel.
PAIRS = [
    ((0, 0), (0, 1)),
    ((1, 0), (0, 2)),
    ((1, 1), (1, 2)),
    ((2, 0), (2, 1)),
    ((3, 0), (2, 2)),
    ((3, 1), (3, 2)),
]
# per-unit (tile-index, base): h0 -> (T1/K1, 0); h1 -> (T1/K1, 64) when it
# is the second unit of its pair, else (T2/K2, 0); h2 -> (T2/K2, 64)
UNIT_LOC = {}
for _pair in PAIRS:
    for _i, _u in enumerate(_pair):
        _h = _u[1]
        if _h == 0:
            UNIT_LOC[_u] = (0, 0)
        elif _h == 2:
            UNIT_LOC[_u] = (1, HD)
        else:
            UNIT_LOC[_u] = (0, HD) if _i == 1 else (1, 0)


def _build_nc():
    nc = bacc.Bacc("TRN2", target_bir_lowering=False, debug=False)
    xT_d = nc.dram_tensor("xT", [D, S], BF16, kind="ExternalInput").ap()
    wqk_d = nc.dram_tensor("wqk", [D, 2 * HPC * HD], BF16, kind="ExternalInput").ap()
    bqk_d = nc.dram_tensor("bqk", [P, 4], F32, kind="ExternalInput").ap()
    wv_d = nc.dram_tensor("wv", [D, HPC * HD], BF16, kind="ExternalInput").ap()
    wp_d = nc.dram_tensor("wp", [HPC * HD, D], BF16, kind="ExternalInput").ap()
    out_d = nc.dram_tensor("out", [S, D], BF16, kind="ExternalOutput").ap()

    with tile.TileContext(nc) as tc, ExitStack() as ctx:
        const = ctx.enter_context(tc.tile_pool(name="const", bufs=1))
        es_pool = ctx.enter_context(tc.tile_pool(name="es", bufs=6))
        ot_pool = ctx.enter_context(tc.tile_pool(name="ot", bufs=2))
        rc_pool = ctx.enter_context(tc.tile_pool(name="rc", bufs=3))
        out_pool = ctx.enter_context(tc.tile_pool(name="outsb", bufs=3))
        ps_s = ctx.enter_context(tc.tile_pool(name="ps_s", bufs=2, space="PSUM"))
        ps_o = ctx.enter_context(tc.tile_pool(name="ps_o", bufs=2, space="PSUM"))
        aux = ctx.enter_context(tc.tile_pool(name="aux", bufs=2, space="PSUM"))

        aux_n = [0]

        def aux_ps():
            aux_n[0] += 1
            return aux.tile([P, QB], F32, tag="aux", name=f"aux{aux_n[0]}")

        # ---- constants / warmup ----
        # ones row at partition HD (=64) feeds the K=1 denominator-broadcast
        # matmul (operands must share the AV psum's denominator-row base
        # partition); partitions 0..63 feed the PE warmup stream. bf16 so the
        # broadcast matmul runs at bf16 rate (fp32 matmuls take the slow
        # LOW/HIGH double pass).
        ones_sb = const.tile([HD + 1, QB], BF16, tag="ones")
        nc.vector.memset(ones_sb[:], 1.0)
        wes = const.tile([1, 16], BF16, tag="wes")
        # dummy exp: forces the ACT table set load at t=0, off the critical path
        nc.scalar.activation(wes[:], ones_sb[0:1, 0:16], EXP)

        warm_n = [0]

        def warm_mm():
            # dependency-free matmul into the (pre-attention idle) ps_o pool;
            # no consumers, so the rotation never blocks. Used to keep the
            # HAM clock gate at 8/8 while input-gated matmuls trickle in.
            warm_n[0] += 1
            wt = ps_o.tile([HD + 1, QB], F32, tag="o", name=f"warm{warm_n[0]}")
            nc.tensor.matmul(
                wt[0:HD, :], ones_sb[0:HD, 0:HD], ones_sb[0:HD, :], start=True, stop=True
            )

        for _ in range(12):
            warm_mm()

        # ---- input loads: direct DMA into the consumed tiles ----
        # x chunks on the ACT HWDGE ring, weights/bias on the SP ring; every
        # matmul input dependency is then a single DMA-queue semaphore carried
        # by the LDWEIGHTS (stationary) or MATMUL (moving) instruction.
        xt = []
        wqk_sb = []
        wv_sb = []
        wp_sb = []
        bqk_sb = const.tile([P, 4], F32, tag="bqk")
        nc.scalar.dma_start(out=bqk_sb[:], in_=bqk_d[:, :])
        # x across all three DMA paths (both HWDGE rings + the GPSIMD SWDGE
        # ring) -- the load phase gates the first exp, so aggregate input
        # bandwidth is the whole ballgame here
        x_engs = [nc.sync, nc.scalar, nc.gpsimd]
        for i in range(KD):
            t = const.tile([P, S], BF16, tag=f"xt{i}")
            x_engs[i % 3].dma_start(out=t[:], in_=xT_d[i * P : (i + 1) * P, :])
            xt.append(t)
        for i in range(KD):
            t = const.tile([P, 2 * HPC * HD], BF16, tag=f"wqk{i}")
            nc.sync.dma_start(out=t[:], in_=wqk_d[i * P : (i + 1) * P, :])
            wqk_sb.append(t)
        for i in range(KD):
            t = const.tile([P, HPC * HD], BF16, tag=f"wv{i}")
            nc.scalar.dma_start(out=t[:], in_=wv_d[i * P : (i + 1) * P, :])
            wv_sb.append(t)
        for h in range(HPC):
            t = const.tile([HD, D], BF16, tag=f"wp{h}")
            nc.scalar.dma_start(out=t[:], in_=wp_d[h * HD : (h + 1) * HD, :])
            wp_sb.append(t)

        # ---- Q^T / K^T / V tiles and emitters ----
        T1 = const.tile([P, S], BF16, tag="T1")  # [q_h0; q_h1]
        T2 = const.tile([P, S], BF16, tag="T2")  # [q_h1 (DMA copy); q_h2]
        K1 = const.tile([P, S], BF16, tag="K1")  # [k_h0; k_h1]
        K2 = const.tile([P, S], BF16, tag="K2")  # [k_h1 (DMA copy); k_h2]
        # spec -> (dst tile, wqk col start, width, bias col, out base)
        # q2/k2 are M=64 matmuls col-tiled to write PSUM partitions 64-127
        # directly, so their DVE bias-add stays partition-aligned at base 64.
        m_specs = {
            "q01": (T1, 0, P, 0, 0),
            "q2": (T2, P, HD, 1, HD),
            "k01": (K1, P + HD, P, 2, 0),
            "k2": (K2, 2 * P + HD, HD, 3, HD),
        }

        def emit_qk(spec, nb):
            dst, c0, w, bcol, ob = m_specs[spec]
            pq = aux_ps()
            tp = (0, ob) if ob else None
            for k in range(KD):
                nc.tensor.matmul(
                    pq[ob : ob + w],
                    wqk_sb[k][:, c0 : c0 + w],
                    xt[k][:, nb * QB : (nb + 1) * QB],
                    start=(k == 0),
                    stop=(k == KD - 1),
                    tile_position=tp,
                )
            nc.vector.tensor_scalar_add(
                dst[ob : ob + w, nb * QB : (nb + 1) * QB],
                pq[ob : ob + w],
                bqk_sb[ob : ob + w, bcol : bcol + 1],
            )

        def copy_row(src_tile, dst_tile, nb):
            # partition-shift q1/k1 rows 64-127 -> 0-63 of the twin tile
            # (SBUF->SBUF DMA; engines can't cross partitions, DMA can)
            nc.gpsimd.dma_start(
                out=dst_tile[0:HD, nb * QB : (nb + 1) * QB],
                in_=src_tile[HD:P, nb * QB : (nb + 1) * QB],
            )

        v1 = []
        for st in range(NKC):
            v1t = const.tile([P, HPC * (HD + 1)], BF16, tag=f"v1_{st}")
            v1.append(v1t)
        VW = HPC * HD  # 192

        def emit_v2(st):
            # V projection for key chunks st and st+1 in one aux-psum use
            pv = aux_ps()
            for j in range(2):
                for k in range(KD):
                    nc.tensor.matmul(
                        pv[:, j * VW : (j + 1) * VW],
                        xt[k][:, (st + j) * P : (st + j + 1) * P],
                        wv_sb[k][:],
                        start=(k == 0),
                        stop=(k == KD - 1),
                    )
            for j in range(2):
                t = v1[st + j]
                nc.gpsimd.memset(t[:], 1.0)
                for h in range(HPC):
                    nc.vector.tensor_copy(
                        t[:, h * (HD + 1) : h * (HD + 1) + HD],
                        pv[:, j * VW + h * HD : j * VW + (h + 1) * HD],
                    )

        qts = [T1, T2]
        kts = [K1, K2]

        # ---- normalize (split) and output projection ----
        raw_map = {}
        ot_map = {}
        out_tiles = {}

        def norm_copy(u, po):
            # one cast-copy frees the AV psum bank; everything downstream is
            # SBUF. bf16 raw costs ~0.4% on the denominators/numerators --
            # well inside the error budget -- and makes the broadcast matmul
            # a fast bf16 one.
            raw = rc_pool.tile(
                [HD + 1, QB], BF16, tag="raw", name=f"raw_{u[0]}_{u[1]}"
            )
            nc.vector.tensor_copy(raw[:], po[:])
            raw_map[u] = raw

        def norm_finish(u):
            qb, h = u
            raw = raw_map.pop(u)
            pr = aux_ps()
            nc.tensor.matmul(
                pr[0:HD, :],
                ones_sb[HD : HD + 1, 0:HD],
                raw[HD : HD + 1, :],
                start=True,
                stop=True,
            )
            rb = rc_pool.tile([HD, QB], F32, tag="rb", name=f"rb_{qb}_{h}")
            nc.vector.reciprocal_approx_fast(rb[:], pr[0:HD, :])
            ot = ot_pool.tile([HD, QB], BF16, tag=f"ot{h}", name=f"ot_{qb}_{h}")
            nc.vector.tensor_mul(ot[:], raw[0:HD, :], rb[:])
            ot_map[u] = ot

        def emit_proj_half(qb, t_i, nb):
            st = qb * (QB // P) + t_i
            if nb == 0:
                out_tiles[st] = out_pool.tile(
                    [P, D], BF16, tag="outsb", name=f"outsb{st}"
                )
            outsb = out_tiles[st]
            pp = aux_ps()
            for h in range(HPC):
                nc.tensor.matmul(
                    pp[:, : D // 2],
                    ot_map[(qb, h)][:, t_i * P : (t_i + 1) * P],
                    wp_sb[h][:, nb * (D // 2) : (nb + 1) * (D // 2)],
                    start=(h == 0),
                    stop=(h == HPC - 1),
                )
            nc.vector.tensor_copy(
                outsb[:, nb * (D // 2) : (nb + 1) * (D // 2)], pp[:, : D // 2]
            )
            if nb == 1:
                # qb3 stores issue after the final exp -- split those across
                # both HWDGE rings to halve the end-of-kernel DMA flush; all
                # earlier stores stay off the exp-saturated ACT ring
                eng = nc.scalar if st >= 12 and st % 2 == 1 else nc.sync
                eng.dma_start(
                    out=out_d[st * P : (st + 1) * P, :], in_=out_tiles.pop(st)[:]
                )

        # ---- static slot schedule (one list of thunks per (pair, group)) ----
        def qk(spec, nb):
            return lambda: emit_qk(spec, nb)

        def vv(st):
            return lambda: emit_v2(st)

        def nf(u):
            return lambda: norm_finish(u)

        def pj(qb, t_i, nb):
            return lambda: emit_proj_half(qb, t_i, nb)

        def cr(src, dst, nb):
            return lambda: copy_row(src, dst, nb)

        slot_plan = [
            [
                [qk("k01", 1), vv(2)],
                [qk("k2", 0), vv(4), cr(K1, K2, 1)],
                [qk("k01", 2), vv(6)],
                [qk("k2", 1), vv(8), cr(K1, K2, 2)],
                [qk("k01", 3), vv(10)],
                [qk("k2", 2), vv(12), cr(K1, K2, 3)],
                [qk("q01", 1), vv(14)],
                [qk("k2", 3), qk("q2", 0), cr(T1, T2, 1)],
            ],
            [
                [qk("q2", 1)],
                [nf((0, 0))],
                [nf((0, 1))],
                [qk("q01", 2)],
                [qk("q2", 2), cr(T1, T2, 2)],
                [qk("q01", 3)],
                [qk("q2", 3), cr(T1, T2, 3)],
                [],
            ],
            [
                [],
                [nf((1, 0))],
                [nf((0, 2))],
                [pj(0, 0, 0)],
                [pj(0, 0, 1)],
                [pj(0, 1, 0)],
                [pj(0, 1, 1)],
                [pj(0, 2, 0)],
            ],
            [
                [pj(0, 2, 1)],
                [pj(0, 3, 0)],
                [pj(0, 3, 1)],
                [nf((1, 1))],
                [nf((1, 2))],
                [pj(1, 0, 0)],
                [pj(1, 0, 1)],
                [pj(1, 1, 0)],
            ],
            [
                [pj(1, 1, 1)],
                [pj(1, 2, 0)],
                [pj(1, 2, 1)],
                [pj(1, 3, 0)],
                [pj(1, 3, 1)],
                [nf((2, 0))],
                [nf((2, 1))],
                [],
            ],
            [
                [nf((3, 0))],
                [nf((2, 2))],
                [pj(2, 0, 0)],
                [pj(2, 0, 1)],
                [pj(2, 1, 0)],
                [pj(2, 1, 1)],
                [pj(2, 2, 0)],
                [pj(2, 2, 1)],
            ],
        ]
        tail_work = [pj(2, 3, 0), pj(2, 3, 1), nf((3, 1)), nf((3, 2))] + [
            pj(3, t_i, nb) for t_i in range(4) for nb in range(2)
        ]

        # ---- pre-attention minimum: K/Q for pair 0 + first V chunks ----
        # Chunk-major interleave of the three input-gated groups, padded with
        # dependency-free warmup matmuls: each arriving x chunk releases a
        # dense burst of PE work, so the HAM clock gate is warm when the
        # final chunk lands and the drain to the first exp runs at 2.4 GHz.
        pq_k = aux_ps()
        pq_q = aux_ps()
        pv0 = ps_s.tile([P, GS * QB], F32, tag="s", name="pv0")
        for k in range(KD):
            st_, stp = k == 0, k == KD - 1
            nc.tensor.matmul(
                pq_k[:],
                wqk_sb[k][:, P + HD : 2 * P + HD],
                xt[k][:, 0:QB],
                start=st_,
                stop=stp,
            )
            nc.tensor.matmul(
                pq_q[:], wqk_sb[k][:, 0:P], xt[k][:, 0:QB], start=st_, stop=stp
            )
            for j in range(2):
                nc.tensor.matmul(
                    pv0[:, j * QB : j * QB + VW],
                    xt[k][:, j * P : (j + 1) * P],
                    wv_sb[k][:],
                    start=st_,
                    stop=stp,
                )
            if k < KD - 1:
                for _ in range(2):
                    warm_mm()
        nc.vector.tensor_scalar_add(K1[:, 0:QB], pq_k[:], bqk_sb[:, 2:3])
        nc.vector.tensor_scalar_add(T1[:, 0:QB], pq_q[:], bqk_sb[:, 0:1])
        for j in range(2):
            t = v1[j]
            nc.gpsimd.memset(t[:], 1.0)
            for h in range(HPC):
                nc.vector.tensor_copy(
                    t[:, h * (HD + 1) : h * (HD + 1) + HD],
                    pv0[:, j * QB + h * HD : j * QB + (h + 1) * HD],
                )
        copy_row(K1, K2, 0)
        copy_row(T1, T2, 0)

        # ---- pairwise attention loop ----
        # Scores for group g are emitted BEFORE the AV matmuls of group g-1,
        # and the two streams' ps_s allocations alternate order each group
        # (A,B | B,A | ...), so with the 2-buffer rotation a stream's scores
        # only wait on an exp that finished two ACT slots earlier: the PE
        # always has runnable score work during the current exp, and ACT
        # never waits on a score matmul that is stuck behind an AV.
        def flush_av(pend):
            order, pom, esm, g, last = pend
            for u in order:
                qb, h = u
                for j in range(GS):
                    kc = g * GS + j
                    nc.tensor.matmul(
                        pom[u][:],
                        v1[kc][:, h * (HD + 1) : (h + 1) * (HD + 1)],
                        esm[u][:, j * QB : (j + 1) * QB],
                        start=(kc == 0),
                        stop=(kc == NKC - 1),
                    )
            if last:
                for u in order:
                    norm_copy(u, pom[u])

        pending = None
        for pi, pair in enumerate(PAIRS):
            pom = {
                u: ps_o.tile([HD + 1, QB], F32, tag="o", name=f"po_{pi}_{i}")
                for i, u in enumerate(pair)
            }
            for g in range(NKC // GS):
                order = list(pair) if g % 2 == 0 else list(pair)[::-1]
                pst = {}
                for u in order:
                    pst[u] = ps_s.tile(
                        [P, GS * QB], F32, tag="s", name=f"s_{pi}_{g}_{u[0]}_{u[1]}"
                    )
                for j in range(GS):
                    kc = g * GS + j
                    for u in order:
                        qb, h = u
                        ti, qo = UNIT_LOC[u]
                        qt, kt = qts[ti], kts[ti]
                        nc.tensor.matmul(
                            pst[u][:, j * QB : (j + 1) * QB],
                            kt[qo : qo + HD, kc * P : (kc + 1) * P],
                            qt[qo : qo + HD, qb * QB : (qb + 1) * QB],
                            start=True,
                            stop=True,
                        )
                esm = {}
                for u in order:
                    es = es_pool.tile(
                        [P, GS * QB], BF16, tag="es", name=f"es_{pi}_{g}_{u[0]}_{u[1]}"
                    )
                    nc.scalar.activation(es[:], pst[u][:], EXP)
                    esm[u] = es
                if pending is not None:
                    flush_av(pending)
                for thunk in slot_plan[pi][g]:
                    thunk()
                pending = (order, pom, esm, g, g == NKC // GS - 1)
        flush_av(pending)
        for thunk in tail_work:
            thunk()

        assert not raw_map

    nc.compile()
    return nc


def get_nc():
    if "nc" not in _CACHE:
        _CACHE["nc"] = _build_nc()
    return _CACHE["nc"]


def shard_inputs(x, w_qkv, b_qkv, w_proj):
    x = np.asarray(x, np.float32)
    w_qkv = np.asarray(w_qkv, np.float32)
    b_qkv = np.asarray(b_qkv, np.float32)
    w_proj = np.asarray(w_proj, np.float32)
    Wq, Wk, Wv = w_qkv[:, :D], w_qkv[:, D : 2 * D], w_qkv[:, 2 * D :]
    bq, bk = b_qkv[:D], b_qkv[D : 2 * D]
    in_maps = []
    for c in range(NCORES):
        b = c // 4
        lo = HD * HPC * (c % 4)
        sl = slice(lo, lo + HPC * HD)
        bq_s = bq[sl] * SCALE
        bk_s = bk[sl]
        bias4 = np.zeros((P, 4), np.float32)
        bias4[:, 0] = bq_s[0:P]
        bias4[HD:, 1] = bq_s[P : P + HD]  # q2 bias-add runs at base partition 64
        bias4[:, 2] = bk_s[0:P]
        bias4[HD:, 3] = bk_s[P : P + HD]  # k2 likewise
        in_maps.append(
            {
                "xT": np.ascontiguousarray(x[b].T.astype(BF16NP)),
                "wqk": np.ascontiguousarray(
                    np.concatenate([Wq[:, sl] * SCALE, Wk[:, sl]], axis=1).astype(
                        BF16NP
                    )
                ),
                "bqk": bias4,
                "wv": np.ascontiguousarray(Wv[:, sl].astype(BF16NP)),
                "wp": np.ascontiguousarray(w_proj[sl, :].astype(BF16NP)),
            }
        )
    return in_maps


def assemble(outs, b_qkv, w_proj, b_proj):
    b_proj = np.asarray(b_proj, np.float32)
    bv = np.asarray(b_qkv, np.float32)[2 * D :]
    b_eff = b_proj + bv @ np.asarray(w_proj, np.float32)
    y = np.empty((B, S, D), np.float32)
    for b in range(B):
        acc = outs[4 * b].astype(np.float32)
        for i in range(1, 4):
            acc = acc + outs[4 * b + i].astype(np.float32)
        y[b] = acc + b_eff
    return y


def run(inputs, trace=False, **kw):
    nc = get_nc()
    in_maps = shard_inputs(
        inputs["x"], inputs["w_qkv"], inputs["b_qkv"], inputs["w_proj"]
    )
    res = run_bass_kernel_spmd(
        nc, in_maps, core_ids=list(range(NCORES)), trace=trace, **kw
    )
    outs = [r["out"] for r in res.results]
    return assemble(outs, inputs["b_qkv"], inputs["w_proj"], inputs["b_proj"]), res


def kernel(x, w_qkv, b_qkv, w_proj, b_proj):
    y, _ = run(
        {"x": x, "w_qkv": w_qkv, "b_qkv": b_qkv, "w_proj": w_proj, "b_proj": b_proj}
    )
    return y


# revision 43
# speedup vs baseline: 1.0163x; 1.0163x over previous
"""Multi-head attention (B=2, S=2048, D=768, H=12) on 8 TRN2 NeuronCores.

Sharding: data-parallel over batch x tensor-parallel over heads.
  core c -> batch c//4, heads 3*(c%4) .. 3*(c%4)+2
Each core computes its 3 heads end-to-end plus the partial output
projection (its 192 rows of w_proj). Host sums the 4 bf16 partials per
batch and adds b_proj + b_v @ w_proj (the V bias is folded out of the
device kernel entirely). No cross-core collectives needed.

Design (v6) — the kernel is an exp-throughput problem: softmax exp is
3*S*S = 12.6M elements on the one ACT engine (~107us), so everything
else is scheduled around keeping ACT saturated from ~25us onward:

  * All inputs host-cast to bf16 and DMA'd DIRECTLY into their SBUF
    tiles (no staging copies): each matmul carries one DMA-queue
    semaphore wait on its LDWEIGHTS (stationary) or MATMUL (moving)
    instruction. x goes via the ACT HWDGE ring, weights via the SP ring.
  * Attention units (qb, h) run pairwise-interleaved; the two streams'
    K=64 score matmuls are emitted adjacently and run concurrently in
    disjoint PE row-groups where their base partitions differ.
  * Everything that is not a score/AV matmul (K/Q/V projections, softmax
    denominator broadcast, output projection halves) is queued into a
    static one-use-per-group slot schedule through a shared 2-buffer
    PSUM pool, so the in-order PE queue never blocks on a slow
    dependency and HAM stays at full clock.
  * Normalize is split: a raw [65,512] PSUM->SBUF copy at pair end frees
    the AV accumulator bank immediately; the broadcast/reciprocal/scale
    runs one pair later from SBUF in its slot.
  * PE warmup matmuls + a dummy activation at t=0 pre-warm the HAM clock
    gate and pre-load the exp table set.

Device kernel layout notes (per core):
  xT   (768, 2048)  = x[b]^T           -> contraction dim on partitions
  wqk  (768, 384)   = [Wq_heads*SCALE | Wk_heads], cols q0 q1 q2 k0 k1 k2
  Q^T/K^T tiles: tq0=[q0;q1] (128p), tq1=[q2] (64p), tk0=[k0;k1], tk1=[k2]
    so each head's Q^T and K^T slices share a base partition (0 or 64),
    which the matmul tile_position inference requires.
  V1   per k-chunk [128, 195]: cols [v_h0 1 v_h1 1 v_h2 1] (ones col per head)
  Scores are computed transposed (k on partitions): S^T[k, qb] so that
  exp(S^T) feeds the AV matmul directly (no transpose of the softmax).
  AV: O'[65, qb] = [V_h|1]^T @ exp(S^T chunk); row 64 = softmax denominator.
  Normalize: recip(denom) broadcast over partitions via a K=1 ones-matmul.
"""

from contextlib import ExitStack

import ml_dtypes
import numpy as np

import concourse.bass as bass
import concourse.mybir as mybir
import concourse.tile as tile
from concourse import bacc
from concourse.bass_utils import run_bass_kernel_spmd

B, S, D = 2, 2048, 768
H, HD = 12, 64
SCALE = HD**-0.5
NCORES = 8
HPC = 3  # heads per core
P = 128
KD = D // P  # 6 chunks of contraction dim for projections
QB = 512  # query block (free dim of score matmuls)
NQB = S // QB  # 4
NKC = S // P  # 16 key chunks
GS = 2  # key chunks per exp group ([128, GS*QB] activation)
F32 = mybir.dt.float32
BF16 = mybir.dt.bfloat16
EXP = mybir.ActivationFunctionType.Exp
BF16NP = ml_dtypes.bfloat16

_CACHE = {}

# pairwise unit schedule: (qb, h) streams A/B per pair. The first unit of
# each pair reads Q^T/K^T at base partition 0, the second at base 64, so
# EVERY pair's K=64 score matmuls run concurrently in disjoint PE row
# groups (2x score throughput). This needs q1/k1 present at both bases:
# T1=[q0;q1], T2=[q1;q2], K1=[k0;k1], K2=[k1;k2], where the q1/k1 rows of
# T2/K2 are partition-shifted copies made by SBUF->SBUF DMA (no PE cost).
# Query blocks complete in order qb0..qb3 so output projections spread
# across the whole kernel.
PAIRS = [
    ((0, 0), (0, 1)),
    ((1, 0), (0, 2)),
    ((1, 1), (1, 2)),
    ((2, 0), (2, 1)),
    ((3, 0), (2, 2)),
    ((3, 1), (3, 2)),
]
# per-unit (tile-index, base): h0 -> (T1/K1, 0); h1 -> (T1/K1, 64) when it
# is the second unit of its pair, else (T2/K2, 0); h2 -> (T2/K2, 64)
UNIT_LOC = {}
for _pair in PAIRS:
    for _i, _u in enumerate(_pair):
        _h = _u[1]
        if _h == 0:
            UNIT_LOC[_u] = (0, 0)
        elif _h == 2:
            UNIT_LOC[_u] = (1, HD)
        else:
            UNIT_LOC[_u] = (0, HD) if _i == 1 else (1, 0)


def _build_nc():
    nc = bacc.Bacc("TRN2", target_bir_lowering=False, debug=False)
    xT_d = nc.dram_tensor("xT", [D, S], BF16, kind="ExternalInput").ap()
    wqk_d = nc.dram_tensor("wqk", [D, 2 * HPC * HD], BF16, kind="ExternalInput").ap()
    bqk_d = nc.dram_tensor("bqk", [P, 4], F32, kind="ExternalInput").ap()
    wv_d = nc.dram_tensor("wv", [D, HPC * HD], BF16, kind="ExternalInput").ap()
    wp_d = nc.dram_tensor("wp", [HPC * HD, D], BF16, kind="ExternalInput").ap()
    out_d = nc.dram_tensor("out", [S, D], BF16, kind="ExternalOutput").ap()

    with tile.TileContext(nc) as tc, ExitStack() as ctx:
        const = ctx.enter_context(tc.tile_pool(name="const", bufs=1))
        es_pool = ctx.enter_context(tc.tile_pool(name="es", bufs=6))
        ot_pool = ctx.enter_context(tc.tile_pool(name="ot", bufs=2))
        rc_pool = ctx.enter_context(tc.tile_pool(name="rc", bufs=3))
        out_pool = ctx.enter_context(tc.tile_pool(name="outsb", bufs=3))
        ps_s = ctx.enter_context(tc.tile_pool(name="ps_s", bufs=2, space="PSUM"))
        ps_o = ctx.enter_context(tc.tile_pool(name="ps_o", bufs=2, space="PSUM"))
        aux = ctx.enter_context(tc.tile_pool(name="aux", bufs=2, space="PSUM"))

        aux_n = [0]

        def aux_ps():
            aux_n[0] += 1
            return aux.tile([P, QB], F32, tag="aux", name=f"aux{aux_n[0]}")

        # ---- constants / warmup ----
        # ones row at partition HD (=64) feeds the K=1 denominator-broadcast
        # matmul (operands must share the AV psum's denominator-row base
        # partition); partitions 0..63 feed the PE warmup stream. bf16 so the
        # broadcast matmul runs at bf16 rate (fp32 matmuls take the slow
        # LOW/HIGH double pass).
        ones_sb = const.tile([HD + 1, QB], BF16, tag="ones")
        nc.vector.memset(ones_sb[:], 1.0)
        wes = const.tile([1, 16], BF16, tag="wes")
        # dummy exp: forces the ACT table set load at t=0, off the critical path
        nc.scalar.activation(wes[:], ones_sb[0:1, 0:16], EXP)

        warm_n = [0]

        def warm_mm():
            # dependency-free matmul into the (pre-attention idle) ps_o pool;
            # no consumers, so the rotation never blocks. Used to keep the
            # HAM clock gate at 8/8 while input-gated matmuls trickle in.
            warm_n[0] += 1
            wt = ps_o.tile([HD + 1, QB], F32, tag="o", name=f"warm{warm_n[0]}")
            nc.tensor.matmul(
                wt[0:HD, :], ones_sb[0:HD, 0:HD], ones_sb[0:HD, :], start=True, stop=True
            )

        for _ in range(12):
            warm_mm()

        # ---- input loads: direct DMA into the consumed tiles ----
        # x chunks on the ACT HWDGE ring, weights/bias on the SP ring; every
        # matmul input dependency is then a single DMA-queue semaphore carried
        # by the LDWEIGHTS (stationary) or MATMUL (moving) instruction.
        xt = []
        wqk_sb = []
        wv_sb = []
        wp_sb = []
        bqk_sb = const.tile([P, 4], F32, tag="bqk")
        nc.scalar.dma_start(out=bqk_sb[:], in_=bqk_d[:, :])
        # x is loaded COLUMN-MAJOR (query/key block 0 of all six contraction
        # chunks first): every pre-attention consumer is column-block local,
        # so the first exp gates on ~1.1 MB instead of the full 4.4 MB of x.
        # Alternate the two HWDGE rings per transfer.
        for i in range(KD):
            t = const.tile([P, S], BF16, tag=f"xt{i}")
            xt.append(t)
        for i in range(KD):
            t = const.tile([P, 2 * HPC * HD], BF16, tag=f"wqk{i}")
            eng = nc.sync if i % 2 == 0 else nc.scalar
            eng.dma_start(out=t[:], in_=wqk_d[i * P : (i + 1) * P, :])
            wqk_sb.append(t)

        def load_x_block(cb):
            for k in range(KD):
                eng = nc.sync if (k + cb) % 2 == 0 else nc.scalar
                eng.dma_start(
                    out=xt[k][:, cb * QB : (cb + 1) * QB],
                    in_=xT_d[k * P : (k + 1) * P, cb * QB : (cb + 1) * QB],
                )

        load_x_block(0)
        for i in range(KD):
            t = const.tile([P, HPC * HD], BF16, tag=f"wv{i}")
            eng = nc.sync if i % 2 == 0 else nc.scalar
            eng.dma_start(out=t[:], in_=wv_d[i * P : (i + 1) * P, :])
            wv_sb.append(t)
        for cb in range(1, NQB):
            load_x_block(cb)
        for h in range(HPC):
            t = const.tile([HD, D], BF16, tag=f"wp{h}")
            nc.scalar.dma_start(out=t[:], in_=wp_d[h * HD : (h + 1) * HD, :])
            wp_sb.append(t)

        # ---- Q^T / K^T / V tiles and emitters ----
        T1 = const.tile([P, S], BF16, tag="T1")  # [q_h0; q_h1]
        T2 = const.tile([P, S], BF16, tag="T2")  # [q_h1 (DMA copy); q_h2]
        K1 = const.tile([P, S], BF16, tag="K1")  # [k_h0; k_h1]
        K2 = const.tile([P, S], BF16, tag="K2")  # [k_h1 (DMA copy); k_h2]
        # spec -> (dst tile, wqk col start, width, bias col, out base)
        # q2/k2 are M=64 matmuls col-tiled to write PSUM partitions 64-127
        # directly, so their DVE bias-add stays partition-aligned at base 64.
        m_specs = {
            "q01": (T1, 0, P, 0, 0),
            "q2": (T2, P, HD, 1, HD),
            "k01": (K1, P + HD, P, 2, 0),
            "k2": (K2, 2 * P + HD, HD, 3, HD),
        }

        def emit_qk(spec, nb):
            dst, c0, w, bcol, ob = m_specs[spec]
            pq = aux_ps()
            tp = (0, ob) if ob else None
            for k in range(KD):
                nc.tensor.matmul(
                    pq[ob : ob + w],
                    wqk_sb[k][:, c0 : c0 + w],
                    xt[k][:, nb * QB : (nb + 1) * QB],
                    start=(k == 0),
                    stop=(k == KD - 1),
                    tile_position=tp,
                )
            nc.vector.tensor_scalar_add(
                dst[ob : ob + w, nb * QB : (nb + 1) * QB],
                pq[ob : ob + w],
                bqk_sb[ob : ob + w, bcol : bcol + 1],
            )

        def copy_row(src_tile, dst_tile, nb):
            # partition-shift q1/k1 rows 64-127 -> 0-63 of the twin tile
            # (SBUF->SBUF DMA; engines can't cross partitions, DMA can)
            nc.gpsimd.dma_start(
                out=dst_tile[0:HD, nb * QB : (nb + 1) * QB],
                in_=src_tile[HD:P, nb * QB : (nb + 1) * QB],
            )

        v1 = []
        for st in range(NKC):
            v1t = const.tile([P, HPC * (HD + 1)], BF16, tag=f"v1_{st}")
            v1.append(v1t)
        VW = HPC * HD  # 192

        def emit_v2(st):
            # V projection for key chunks st and st+1 in one aux-psum use
            pv = aux_ps()
            for j in range(2):
                for k in range(KD):
                    nc.tensor.matmul(
                        pv[:, j * VW : (j + 1) * VW],
                        xt[k][:, (st + j) * P : (st + j + 1) * P],
                        wv_sb[k][:],
                        start=(k == 0),
                        stop=(k == KD - 1),
                    )
            for j in range(2):
                t = v1[st + j]
                nc.gpsimd.memset(t[:], 1.0)
                for h in range(HPC):
                    nc.vector.tensor_copy(
                        t[:, h * (HD + 1) : h * (HD + 1) + HD],
                        pv[:, j * VW + h * HD : j * VW + (h + 1) * HD],
                    )

        qts = [T1, T2]
        kts = [K1, K2]

        # ---- normalize (split) and output projection ----
        raw_map = {}
        ot_map = {}
        out_tiles = {}

        def norm_copy(u, po):
            # one cast-copy frees the AV psum bank; everything downstream is
            # SBUF. bf16 raw costs ~0.4% on the denominators/numerators --
            # well inside the error budget -- and makes the broadcast matmul
            # a fast bf16 one.
            raw = rc_pool.tile(
                [HD + 1, QB], BF16, tag="raw", name=f"raw_{u[0]}_{u[1]}"
            )
            nc.vector.tensor_copy(raw[:], po[:])
            raw_map[u] = raw

        def norm_finish(u):
            qb, h = u
            raw = raw_map.pop(u)
            pr = aux_ps()
            nc.tensor.matmul(
                pr[0:HD, :],
                ones_sb[HD : HD + 1, 0:HD],
                raw[HD : HD + 1, :],
                start=True,
                stop=True,
            )
            rb = rc_pool.tile([HD, QB], F32, tag="rb", name=f"rb_{qb}_{h}")
            nc.vector.reciprocal_approx_fast(rb[:], pr[0:HD, :])
            ot = ot_pool.tile([HD, QB], BF16, tag=f"ot{h}", name=f"ot_{qb}_{h}")
            nc.vector.tensor_mul(ot[:], raw[0:HD, :], rb[:])
            ot_map[u] = ot

        def emit_proj_half(qb, t_i, nb):
            st = qb * (QB // P) + t_i
            if nb == 0:
                out_tiles[st] = out_pool.tile(
                    [P, D], BF16, tag="outsb", name=f"outsb{st}"
                )
            outsb = out_tiles[st]
            pp = aux_ps()
            for h in range(HPC):
                nc.tensor.matmul(
                    pp[:, : D // 2],
                    ot_map[(qb, h)][:, t_i * P : (t_i + 1) * P],
                    wp_sb[h][:, nb * (D // 2) : (nb + 1) * (D // 2)],
                    start=(h == 0),
                    stop=(h == HPC - 1),
                )
            nc.vector.tensor_copy(
                outsb[:, nb * (D // 2) : (nb + 1) * (D // 2)], pp[:, : D // 2]
            )
            if nb == 1:
                # qb3 stores issue after the final exp -- split those across
                # both HWDGE rings to halve the end-of-kernel DMA flush; all
                # earlier stores stay off the exp-saturated ACT ring
                eng = nc.scalar if st >= 12 and st % 2 == 1 else nc.sync
                eng.dma_start(
                    out=out_d[st * P : (st + 1) * P, :], in_=out_tiles.pop(st)[:]
                )

        # ---- static slot schedule (one list of thunks per (pair, group)) ----
        def qk(spec, nb):
            return lambda: emit_qk(spec, nb)

        def vv(st):
            return lambda: emit_v2(st)

        def nf(u):
            return lambda: norm_finish(u)

        def pj(qb, t_i, nb):
            return lambda: emit_proj_half(qb, t_i, nb)

        def cr(src, dst, nb):
            return lambda: copy_row(src, dst, nb)

        slot_plan = [
            [
                [qk("k01", 1), vv(2)],
                [qk("k2", 0), vv(4), cr(K1, K2, 1)],
                [qk("k01", 2), vv(6)],
                [qk("k2", 1), vv(8), cr(K1, K2, 2)],
                [qk("k01", 3), vv(10)],
                [qk("k2", 2), vv(12), cr(K1, K2, 3)],
                [qk("q01", 1), vv(14)],
                [qk("k2", 3), qk("q2", 0), cr(T1, T2, 1)],
            ],
            [
                [qk("q2", 1)],
                [nf((0, 0))],
                [nf((0, 1))],
                [qk("q01", 2)],
                [qk("q2", 2), cr(T1, T2, 2)],
                [qk("q01", 3)],
                [qk("q2", 3), cr(T1, T2, 3)],
                [],
            ],
            [
                [],
                [nf((1, 0))],
                [nf((0, 2))],
                [pj(0, 0, 0)],
                [pj(0, 0, 1)],
                [pj(0, 1, 0)],
                [pj(0, 1, 1)],
                [pj(0, 2, 0)],
            ],
            [
                [pj(0, 2, 1)],
                [pj(0, 3, 0)],
                [pj(0, 3, 1)],
                [nf((1, 1))],
                [nf((1, 2))],
                [pj(1, 0, 0)],
                [pj(1, 0, 1)],
                [pj(1, 1, 0)],
            ],
            [
                [pj(1, 1, 1)],
                [pj(1, 2, 0)],
                [pj(1, 2, 1)],
                [pj(1, 3, 0)],
                [pj(1, 3, 1)],
                [nf((2, 0))],
                [nf((2, 1))],
                [],
            ],
            [
                [nf((3, 0))],
                [nf((2, 2))],
                [pj(2, 0, 0)],
                [pj(2, 0, 1)],
                [pj(2, 1, 0)],
                [pj(2, 1, 1)],
                [pj(2, 2, 0)],
                [pj(2, 2, 1)],
            ],
        ]
        tail_work = [pj(2, 3, 0), pj(2, 3, 1), nf((3, 1)), nf((3, 2))] + [
            pj(3, t_i, nb) for t_i in range(4) for nb in range(2)
        ]

        # ---- pre-attention minimum: K/Q for pair 0 + first V chunks ----
        # Chunk-major interleave of the three input-gated groups, padded with
        # dependency-free warmup matmuls: each arriving x chunk releases a
        # dense burst of PE work, so the HAM clock gate is warm when the
        # final chunk lands and the drain to the first exp runs at 2.4 GHz.
        pq_k = aux_ps()
        pq_q = aux_ps()
        pv0 = ps_s.tile([P, GS * QB], F32, tag="s", name="pv0")
        for k in range(KD):
            st_, stp = k == 0, k == KD - 1
            nc.tensor.matmul(
                pq_k[:],
                wqk_sb[k][:, P + HD : 2 * P + HD],
                xt[k][:, 0:QB],
                start=st_,
                stop=stp,
            )
            nc.tensor.matmul(
                pq_q[:], wqk_sb[k][:, 0:P], xt[k][:, 0:QB], start=st_, stop=stp
            )
            if k < KD - 1:
                for _ in range(2):
                    warm_mm()
        # V chunks 0/1 after the K/Q matmuls: wv lands a little later than
        # x block 0, and these must not head-of-line-block the first scores
        for k in range(KD):
            for j in range(2):
                nc.tensor.matmul(
                    pv0[:, j * QB : j * QB + VW],
                    xt[k][:, j * P : (j + 1) * P],
                    wv_sb[k][:],
                    start=(k == 0),
                    stop=(k == KD - 1),
                )
        nc.vector.tensor_scalar_add(K1[:, 0:QB], pq_k[:], bqk_sb[:, 2:3])
        nc.vector.tensor_scalar_add(T1[:, 0:QB], pq_q[:], bqk_sb[:, 0:1])
        for j in range(2):
            t = v1[j]
            nc.gpsimd.memset(t[:], 1.0)
            for h in range(HPC):
                nc.vector.tensor_copy(
                    t[:, h * (HD + 1) : h * (HD + 1) + HD],
                    pv0[:, j * QB + h * HD : j * QB + (h + 1) * HD],
                )
        copy_row(K1, K2, 0)
        copy_row(T1, T2, 0)

        # ---- pairwise attention loop ----
        # Scores for group g are emitted BEFORE the AV matmuls of group g-1,
        # and the two streams' ps_s allocations alternate order each group
        # (A,B | B,A | ...), so with the 2-buffer rotation a stream's scores
        # only wait on an exp that finished two ACT slots earlier: the PE
        # always has runnable score work during the current exp, and ACT
        # never waits on a score matmul that is stuck behind an AV.
        def flush_av(pend):
            order, pom, esm, g, last = pend
            for u in order:
                qb, h = u
                for j in range(GS):
                    kc = g * GS + j
                    nc.tensor.matmul(
                        pom[u][:],
                        v1[kc][:, h * (HD + 1) : (h + 1) * (HD + 1)],
                        esm[u][:, j * QB : (j + 1) * QB],
                        start=(kc == 0),
                        stop=(kc == NKC - 1),
                    )
            if last:
                for u in order:
                    norm_copy(u, pom[u])

        pending = None
        for pi, pair in enumerate(PAIRS):
            pom = {
                u: ps_o.tile([HD + 1, QB], F32, tag="o", name=f"po_{pi}_{i}")
                for i, u in enumerate(pair)
            }
            for g in range(NKC // GS):
                order = list(pair) if g % 2 == 0 else list(pair)[::-1]
                pst = {}
                for u in order:
                    pst[u] = ps_s.tile(
                        [P, GS * QB], F32, tag="s", name=f"s_{pi}_{g}_{u[0]}_{u[1]}"
                    )
                for j in range(GS):
                    kc = g * GS + j
                    for u in order:
                        qb, h = u
                        ti, qo = UNIT_LOC[u]
                        qt, kt = qts[ti], kts[ti]
                        nc.tensor.matmul(
                            pst[u][:, j * QB : (j + 1) * QB],
                            kt[qo : qo + HD, kc * P : (kc + 1) * P],
                            qt[qo : qo + HD, qb * QB : (qb + 1) * QB],
                            start=True,
                            stop=True,
                        )
                esm = {}
                for u in order:
                    es = es_pool.tile(
                        [P, GS * QB], BF16, tag="es", name=f"es_{pi}_{g}_{u[0]}_{u[1]}"
                    )
                    nc.scalar.activation(es[:], pst[u][:], EXP)
                    esm[u] = es
                if pending is not None:
                    flush_av(pending)
                for thunk in slot_plan[pi][g]:
                    thunk()
                pending = (order, pom, esm, g, g == NKC // GS - 1)
        flush_av(pending)
        for thunk in tail_work:
            thunk()

        assert not raw_map

    nc.compile()
    return nc


def get_nc():
    if "nc" not in _CACHE:
        _CACHE["nc"] = _build_nc()
    return _CACHE["nc"]


def shard_inputs(x, w_qkv, b_qkv, w_proj):
    x = np.asarray(x, np.float32)
    w_qkv = np.asarray(w_qkv, np.float32)
    b_qkv = np.asarray(b_qkv, np.float32)
    w_proj = np.asarray(w_proj, np.float32)
    Wq, Wk, Wv = w_qkv[:, :D], w_qkv[:, D : 2 * D], w_qkv[:, 2 * D :]
    bq, bk = b_qkv[:D], b_qkv[D : 2 * D]
    in_maps = []
    for c in range(NCORES):
        b = c // 4
        lo = HD * HPC * (c % 4)
        sl = slice(lo, lo + HPC * HD)
        bq_s = bq[sl] * SCALE
        bk_s = bk[sl]
        bias4 = np.zeros((P, 4), np.float32)
        bias4[:, 0] = bq_s[0:P]
        bias4[HD:, 1] = bq_s[P : P + HD]  # q2 bias-add runs at base partition 64
        bias4[:, 2] = bk_s[0:P]
        bias4[HD:, 3] = bk_s[P : P + HD]  # k2 likewise
        in_maps.append(
            {
                "xT": np.ascontiguousarray(x[b].T.astype(BF16NP)),
                "wqk": np.ascontiguousarray(
                    np.concatenate([Wq[:, sl] * SCALE, Wk[:, sl]], axis=1).astype(
                        BF16NP
                    )
                ),
                "bqk": bias4,
                "wv": np.ascontiguousarray(Wv[:, sl].astype(BF16NP)),
                "wp": np.ascontiguousarray(w_proj[sl, :].astype(BF16NP)),
            }
        )
    return in_maps


def assemble(outs, b_qkv, w_proj, b_proj):
    b_proj = np.asarray(b_proj, np.float32)
    bv = np.asarray(b_qkv, np.float32)[2 * D :]
    b_eff = b_proj + bv @ np.asarray(w_proj, np.float32)
    y = np.empty((B, S, D), np.float32)
    for b in range(B):
        acc = outs[4 * b].astype(np.float32)
        for i in range(1, 4):
            acc = acc + outs[4 * b + i].astype(np.float32)
        y[b] = acc + b_eff
    return y


def run(inputs, trace=False, **kw):
    nc = get_nc()
    in_maps = shard_inputs(
        inputs["x"], inputs["w_qkv"], inputs["b_qkv"], inputs["w_proj"]
    )
    res = run_bass_kernel_spmd(
        nc, in_maps, core_ids=list(range(NCORES)), trace=trace, **kw
    )
    outs = [r["out"] for r in res.results]
    return assemble(outs, inputs["b_qkv"], inputs["w_proj"], inputs["b_proj"]), res


def kernel(x, w_qkv, b_qkv, w_proj, b_proj):
    y, _ = run(
        {"x": x, "w_qkv": w_qkv, "b_qkv": b_qkv, "w_proj": w_proj, "b_proj": b_proj}
    )
    return y
